# revision 23
# baseline (speedup 1.0000x reference)
"""Multi-head causal attention (B=4, T=2048, N=1024, H=16) on 8 TRN2 NeuronCores.

Sharding: core c = (batch b = c//2, head-group hg = c%2). Each core computes
full-T causal attention for its 8 heads of its batch, plus the partial output
projection for its head rows. Host sums the two head-group partials per batch
and adds b_proj (and the v-bias contribution, which is exact because softmax
rows sum to 1).

Per-core kernel, one globally-scheduled instruction stream (no phase
barriers) so the Tile scheduler can fill ACT-paced attention gaps with
qkv-projection and output-projection matmuls:

  qkv chunk tck (t-cols tck*512):  qkT[j,t] = Wqk.T @ x.T (bias added on
  DVE), v[t,d] = x @ Wv stored bf16 as [64 v | 1] x 8 heads.

  attention (qc, hp): scores TRANSPOSED [k-part, q-free]; the two heads of
  the pair run CONCURRENTLY in the PE array via row-tiling (lhsT at
  partitions 0:64 / 64:128 -> tile_position (0,0)/(64,0)), writing the two
  halves of one [128, 2, 512] PSUM tile.  One fused 1024-wide exp (ACT,
  scale=1/8) covers both heads; diagonal blocks are column-trimmed (the
  fully-masked prefix is neither computed nor exp'd) and only the 128-col
  boundary block is mask-multiplied (bf16 DVE).  The av matmul
  lhsT = v[128,65] (64 v cols + ones) accumulates y.T AND the softmax row
  sums into PSUM [65, 512].  Normalize: DVE reciprocal of the sums row,
  gpsimd partition-broadcast, DVE multiply into the projection-ready yT
  tile.  Projection: yT.T @ Wp in f32r, DMA'd straight from PSUM.

Emission order qkv(0), att(0), qkv(1), proj(0), att(1), ... lets the
dependency scheduler overlap everything; PE stream work (~496k cycles) is
the target critical path.
"""
import numpy as np
import ml_dtypes
from contextlib import ExitStack

import concourse.bass as bass
import concourse.mybir as mybir
from concourse import bacc
from concourse import bass_utils as _bu
from concourse.bass_utils import run_bass_kernel_spmd
from concourse.tile import TileContext


F32 = mybir.dt.float32
F32R = mybir.dt.float32r
BF16 = mybir.dt.bfloat16
AF = mybir.ActivationFunctionType

B, T, N, H = 4, 2048, 1024, 16
Dh = 64
HG = 512            # head-group width per core (8 heads x 64)
NCORES = 8
KT = N // 128       # 8 contraction tiles for qkv
NQC = T // 512      # 4 q-chunks
NVT = T // 128      # 16 v tiles / k blocks

_CACHE = {}
_last_results = None


def _build():
    if "nc" in _CACHE:
        return _CACHE["nc"]

    nc = bacc.Bacc("TRN2", target_bir_lowering=False)

    xT = nc.declare_dram_parameter("xT", [N, T], BF16, isOutput=False)
    wqk = nc.declare_dram_parameter("wqk", [N, 2 * HG], BF16, isOutput=False)
    wv = nc.declare_dram_parameter("wv", [N, HG], BF16, isOutput=False)
    wp = nc.declare_dram_parameter("wp", [HG, N], F32R, isOutput=False)
    bqk = nc.declare_dram_parameter("bqk", [2 * HG, 1], F32, isOutput=False)
    tri = nc.declare_dram_parameter("tri", [128, 128], BF16, isOutput=False)
    out = nc.declare_dram_parameter("out", [T, N], F32, isOutput=True)

    with TileContext(nc) as tc, ExitStack() as st:
        consts = st.enter_context(tc.tile_pool(name="consts", bufs=1))
        wpool = st.enter_context(tc.tile_pool(name="wpool", bufs=1))
        xpool = st.enter_context(tc.tile_pool(name="xpool", bufs=2))
        resid = st.enter_context(tc.tile_pool(name="resid", bufs=1))
        attp = st.enter_context(tc.tile_pool(name="attp", bufs=3))
        smallp = st.enter_context(tc.tile_pool(name="smallp", bufs=2))
        ytp = st.enter_context(tc.tile_pool(name="ytp", bufs=4))
        outp = st.enter_context(tc.tile_pool(name="outp", bufs=2))
        spool = st.enter_context(tc.tile_pool(name="spool", bufs=2,
                                              space="PSUM"))
        ypool = st.enter_context(tc.tile_pool(name="ypool", bufs=1,
                                              space="PSUM"))
        fillps = st.enter_context(tc.tile_pool(name="fillps", bufs=2,
                                               space="PSUM"))

        # DMA order: x chunk-0 / wqk pairs first so the first qk matmul can
        # start after one pair lands; wv next (v matmuls), wp (proj) last.
        # Startup DMAs split across both HWDGE queues (sync + scalar): the
        # ACT queue is idle until the first exp anyway, and two queues halve
        # the descriptor-issue serialization.
        xt0 = []
        wqk_t = []
        for k in range(KT):
            t = xpool.tile([128, 512], BF16, name=f"xt0_{k}", tag=f"x{k}")
            nc.scalar.dma_start(out=t, in_=xT[k * 128:(k + 1) * 128, 0:512])
            xt0.append(t)
            t = wpool.tile([128, 2 * HG], BF16, name=f"wqk{k}", tag=f"wqk{k}")
            nc.sync.dma_start(out=t, in_=wqk[k * 128:(k + 1) * 128, :])
            wqk_t.append(t)
        tri_sb = consts.tile([128, 128], BF16, name="tri_sb", tag="tri")
        nc.sync.dma_start(out=tri_sb, in_=tri[:, :])
        bqk_sb = []
        for j in range(8):
            t = consts.tile([128, 1], F32, name=f"bqk_sb{j}", tag=f"bqk{j}")
            nc.sync.dma_start(out=t, in_=bqk[j * 128:(j + 1) * 128, :])
            bqk_sb.append(t)
        wv_t = []
        for k in range(KT):
            t = wpool.tile([128, HG], BF16, name=f"wv{k}", tag=f"wv{k}")
            nc.scalar.dma_start(out=t, in_=wv[k * 128:(k + 1) * 128, :])
            wv_t.append(t)
        wp_t = []
        for j in range(4):
            t = wpool.tile([128, N], F32R, name=f"wp{j}", tag=f"wp{j}")
            nc.sync.dma_start(out=t, in_=wp[j * 128:(j + 1) * 128, :])
            wp_t.append(t)

        # residents: qT/kT per head-pair tile [128, T]; v per 128-row chunk,
        # bf16, layout [64 v-cols + ones] x 8 heads = 520 cols
        qT = [resid.tile([128, T], BF16, name=f"qT{j}", tag=f"qT{j}")
              for j in range(4)]
        kTt = [resid.tile([128, T], BF16, name=f"kT{j}", tag=f"kT{j}")
               for j in range(4)]
        vt = [resid.tile([128, 8, 65], BF16, name=f"v{m}", tag=f"v{m}")
              for m in range(NVT)]

        def qkv_units(tck):
            """Filler closures for qkv chunk tck: x DMAs, 8 qk groups,
            4 v groups — emitted one unit at a time between att iters."""
            c0 = tck * 512
            xt = []

            def u_dma():
                for k in range(KT):
                    if tck == 0:
                        xt.append(xt0[k])
                        continue
                    t = xpool.tile([128, 512], BF16, name=f"xt{tck}_{k}",
                                   tag=f"x{k}")
                    nc.sync.dma_start(out=t, in_=xT[k * 128:(k + 1) * 128,
                                                    c0:c0 + 512])
                    xt.append(t)

            def u_qk(jc):
                def f():
                    ps = fillps.tile([128, 512], F32,
                                     name=f"qk_ps{tck}_{jc}", tag="fill")
                    for k in range(KT):
                        nc.tensor.matmul(
                            ps,
                            wqk_t[k][:, jc * 128:(jc + 1) * 128],
                            xt[k],
                            start=(k == 0), stop=(k == KT - 1))
                    dst = (qT[jc] if jc < 4 else kTt[jc - 4])
                    nc.vector.tensor_scalar_add(dst[:, c0:c0 + 512], ps,
                                                bqk_sb[jc])
                return f

            def u_v(mc):
                def f():
                    ps = fillps.tile([128, 512], F32,
                                     name=f"v_ps{tck}_{mc}", tag="fill")
                    for k in range(KT):
                        nc.tensor.matmul(
                            ps,
                            xt[k][:, mc * 128:(mc + 1) * 128],
                            wv_t[k],
                            start=(k == 0), stop=(k == KT - 1))
                    dst = vt[tck * 4 + mc]
                    nc.vector.tensor_copy(
                        dst[:, :, 0:64],
                        ps.rearrange("p (h c) -> p h c", c=64))
                    nc.vector.memset(dst[:, :, 64:65], 1.0)
                return f

            units = [u_dma]
            units += [u_qk(jc) for jc in (0, 4, 1, 5, 2, 6, 3, 7)]
            units += [u_v(mc) for mc in range(4)]
            return units

        def emit_att(qc, filler):
            """Emit attention for q-chunk qc, interleaving the filler unit
            closures evenly across the (hp, kb) iterations."""
            qcol = qc * 512
            nkb = 4 * qc + 4
            n_iters = 4 * nkb
            # unit k fires after iteration floor(k * n_iters / n_units)
            fire_at = {}
            for k in range(len(filler)):
                fire_at.setdefault(k * n_iters // max(1, len(filler)),
                                   []).append(filler[k])
            it = 0
            yts = []
            for hp in range(4):
                y_ps = [ypool.tile([65, 512], F32,
                                   name=f"y_ps{qc}_{hp}_{h01}",
                                   tag=f"y{h01}")
                        for h01 in range(2)]
                for kb in range(nkb):
                    m = kb - 4 * qc          # >= 0 on diagonal blocks
                    lo = m * 128 if m > 0 else 0
                    s_ps = spool.tile([128, 2, 512], F32,
                                      name=f"s{qc}_{hp}_{kb}", tag="s")
                    for h01 in range(2):
                        hb = h01 * 64
                        nc.tensor.matmul(
                            s_ps[:, h01, lo:512],
                            kTt[hp][hb:hb + 64,
                                    kb * 128:(kb + 1) * 128],
                            qT[hp][hb:hb + 64, qcol + lo:qcol + 512],
                            start=True, stop=True)
                    attT = attp.tile([128, 2, 512], BF16,
                                     name=f"attT{qc}_{hp}_{kb}", tag="attT")
                    nc.scalar.activation(out=attT[:, :, lo:512],
                                         in_=s_ps[:, :, lo:512],
                                         func=AF.Exp, scale=0.125)
                    if m >= 0:
                        for h01 in range(2):
                            nc.vector.tensor_mul(attT[:, h01, lo:lo + 128],
                                                 attT[:, h01, lo:lo + 128],
                                                 tri_sb)
                    for h01 in range(2):
                        # av trimmed to the unmasked columns; the masked
                        # prefix was fully written by earlier (lower-kb)
                        # accumulation steps, so group bookkeeping is off.
                        nc.tensor.matmul(
                            y_ps[h01][:, lo:512],
                            vt[kb][:, 2 * hp + h01, :],
                            attT[:, h01, lo:512],
                            start=(kb == 0), stop=(kb == nkb - 1),
                            skip_group_check=True)
                    for f in fire_at.get(it, ()):
                        f()
                    it += 1
                yt = ytp.tile([128, 512], F32R, name=f"yt{qc}_{hp}",
                              tag=f"yt{hp}")
                for h01 in range(2):
                    srow = smallp.tile([1, 512], F32,
                                       name=f"srow{qc}_{hp}_{h01}",
                                       tag=f"srow{h01}")
                    nc.vector.tensor_copy(srow, y_ps[h01][64:65, :])
                    bcst = smallp.tile([64, 512], F32,
                                       name=f"bc{qc}_{hp}_{h01}",
                                       tag=f"bc{h01}")
                    nc.gpsimd.partition_broadcast(bcst, srow)
                    nc.vector.reciprocal_approx_fast(out=bcst, in_=bcst)
                    nc.vector.tensor_mul(
                        yt[h01 * 64:(h01 + 1) * 64, :],
                        y_ps[h01][0:64, :], bcst)
                yts.append(yt)
            return yts

        def proj_units(qc, yts):
            """8 filler closures (one per qb/nch) projecting q-chunk qc."""
            qcol = qc * 512
            o_sb = {}

            def u(qb, nch):
                def f():
                    if nch == 0:
                        o_sb[qb] = outp.tile([128, N], F32,
                                             name=f"o{qc}_{qb}", tag="o")
                    p_ps = fillps.tile([128, 512], F32,
                                       name=f"p{qc}_{qb}_{nch}", tag="fill")
                    for hp in range(4):
                        nc.tensor.matmul(
                            p_ps,
                            yts[hp][:, qb * 128:(qb + 1) * 128],
                            wp_t[hp][:, nch * 512:(nch + 1) * 512],
                            start=(hp == 0), stop=(hp == 3))
                    nc.vector.tensor_copy(
                        o_sb[qb][:, nch * 512:(nch + 1) * 512], p_ps)
                    if nch == 1:
                        nc.sync.dma_start(
                            out=out[qcol + qb * 128:qcol + (qb + 1) * 128,
                                    :],
                            in_=o_sb[qb])
                return f

            return [u(qb, nch) for qb in range(4) for nch in range(2)]

        # qkv(0) runs undiluted up front (ACT has nothing to do yet);
        # qkv(qc+1) spreads through att(qc); the projections of qc 0..2
        # spread through att(3); proj(3) is the tail.
        for u in qkv_units(0):
            u()
        pend_proj = []
        for qc in range(NQC):
            filler = qkv_units(qc + 1) if qc < NQC - 1 else pend_proj
            yts = emit_att(qc, filler)
            pend_proj.extend(proj_units(qc, yts))
        for u in pend_proj[-8:]:
            u()

    nc.compile()
    _CACHE["nc"] = nc
    return nc


def kernel(x, W_attn, b_attn, W_proj, b_proj):
    global _last_results
    nc = _build()

    x = np.asarray(x, dtype=np.float32)
    W_attn = np.asarray(W_attn, dtype=np.float32)
    b_attn = np.asarray(b_attn, dtype=np.float32)
    W_proj = np.asarray(W_proj, dtype=np.float32)
    b_proj = np.asarray(b_proj, dtype=np.float32)

    kk = np.arange(128)[:, None]
    jj = np.arange(128)[None, :]
    tri_np = (jj >= kk).astype(ml_dtypes.bfloat16)

    in_maps = []
    for c in range(NCORES):
        b, hg = divmod(c, 2)
        s = hg * HG
        xT_c = np.ascontiguousarray(x[b].T).astype(ml_dtypes.bfloat16)
        wqk_c = np.ascontiguousarray(
            np.concatenate([W_attn[:, s:s + HG],
                            W_attn[:, N + s:N + s + HG]],
                           axis=1)).astype(ml_dtypes.bfloat16)
        wv_c = np.ascontiguousarray(
            W_attn[:, 2 * N + s:2 * N + s + HG]).astype(ml_dtypes.bfloat16)
        wp_c = np.ascontiguousarray(W_proj[s:s + HG, :])
        bqk_c = np.ascontiguousarray(
            np.concatenate([b_attn[s:s + HG],
                            b_attn[N + s:N + s + HG]]).reshape(2 * HG, 1))
        in_maps.append({
            "xT": xT_c, "wqk": wqk_c, "wv": wv_c, "wp": wp_c,
            "bqk": bqk_c, "tri": tri_np,
        })

    res = run_bass_kernel_spmd(nc, in_maps, list(range(NCORES)))
    _last_results = res
    outs = [res.results[c]["out"] for c in range(NCORES)]
    # v-bias: softmax rows sum to 1, so att @ (xWv + bv) = att @ (xWv) + bv;
    # its projection (bv @ W_proj) plus b_proj are added here, exactly.
    bv = b_attn[2 * N:3 * N]
    extra = bv @ W_proj + b_proj
    y = np.stack([outs[2 * b] + outs[2 * b + 1] for b in range(B)])
    return (y + extra[None, None, :]).astype(np.float32)


# revision 24
# speedup vs baseline: 1.1823x; 1.1823x over previous
"""Multi-head causal attention (B=4, T=2048, N=1024, H=16) on 8 TRN2 NeuronCores.

Sharding: core c = (batch b = c//2, head-group hg = c%2). Each core computes
full-T causal attention for its 8 heads of its batch, plus the partial output
projection for its head rows. Host sums the two head-group partials per batch
and adds b_proj (and the v-bias contribution, which is exact because softmax
rows sum to 1).

Per-core kernel, one globally-scheduled instruction stream (no phase
barriers) so the Tile scheduler can fill ACT-paced attention gaps with
qkv-projection and output-projection matmuls:

  qkv chunk tck (t-cols tck*512):  qkT[j,t] = Wqk.T @ x.T (bias added on
  DVE), v[t,d] = x @ Wv stored bf16 as [64 v | 1] x 8 heads.

  attention (qc, hp): scores TRANSPOSED [k-part, q-free]; the two heads of
  the pair run CONCURRENTLY in the PE array via row-tiling (lhsT at
  partitions 0:64 / 64:128 -> tile_position (0,0)/(64,0)), writing the two
  halves of one [128, 2, 512] PSUM tile.  One fused 1024-wide exp (ACT,
  scale=1/8) covers both heads; diagonal blocks are column-trimmed (the
  fully-masked prefix is neither computed nor exp'd) and only the 128-col
  boundary block is mask-multiplied (bf16 DVE).  The av matmul
  lhsT = v[128,65] (64 v cols + ones) accumulates y.T AND the softmax row
  sums into PSUM [65, 512].  Normalize: DVE reciprocal of the sums row,
  gpsimd partition-broadcast, DVE multiply into the projection-ready yT
  tile.  Projection: yT.T @ Wp in f32r, DMA'd straight from PSUM.

Emission order qkv(0), att(0), qkv(1), proj(0), att(1), ... lets the
dependency scheduler overlap everything; PE stream work (~496k cycles) is
the target critical path.
"""
import numpy as np
import ml_dtypes
from contextlib import ExitStack

import concourse.bass as bass
import concourse.mybir as mybir
from concourse import bacc
from concourse import bass_utils as _bu
from concourse.bass_utils import run_bass_kernel_spmd
from concourse.tile import TileContext


F32 = mybir.dt.float32
F32R = mybir.dt.float32r
BF16 = mybir.dt.bfloat16
AF = mybir.ActivationFunctionType

B, T, N, H = 4, 2048, 1024, 16
Dh = 64
HG = 512            # head-group width per core (8 heads x 64)
NCORES = 8
KT = N // 128       # 8 contraction tiles for qkv
NQC = T // 512      # 4 q-chunks
NVT = T // 128      # 16 v tiles / k blocks

_CACHE = {}
_last_results = None


def _build():
    if "nc" in _CACHE:
        return _CACHE["nc"]

    nc = bacc.Bacc("TRN2", target_bir_lowering=False)

    xT = nc.declare_dram_parameter("xT", [N, T], BF16, isOutput=False)
    wqk = nc.declare_dram_parameter("wqk", [N, 2 * HG], BF16, isOutput=False)
    wv = nc.declare_dram_parameter("wv", [N, HG], BF16, isOutput=False)
    wp = nc.declare_dram_parameter("wp", [HG, N], F32R, isOutput=False)
    bqk = nc.declare_dram_parameter("bqk", [2 * HG, 1], F32, isOutput=False)
    tri = nc.declare_dram_parameter("tri", [128, 128], BF16, isOutput=False)
    out = nc.declare_dram_parameter("out", [T, N], F32, isOutput=True)

    with TileContext(nc) as tc, ExitStack() as st:
        consts = st.enter_context(tc.tile_pool(name="consts", bufs=1))
        wpool = st.enter_context(tc.tile_pool(name="wpool", bufs=1))
        xpool = st.enter_context(tc.tile_pool(name="xpool", bufs=2))
        resid = st.enter_context(tc.tile_pool(name="resid", bufs=1))
        attp = st.enter_context(tc.tile_pool(name="attp", bufs=3))
        smallp = st.enter_context(tc.tile_pool(name="smallp", bufs=2))
        ytp = st.enter_context(tc.tile_pool(name="ytp", bufs=4))
        outp = st.enter_context(tc.tile_pool(name="outp", bufs=2))
        spool = st.enter_context(tc.tile_pool(name="spool", bufs=2,
                                              space="PSUM"))
        ypool = st.enter_context(tc.tile_pool(name="ypool", bufs=1,
                                              space="PSUM"))
        fillps = st.enter_context(tc.tile_pool(name="fillps", bufs=2,
                                               space="PSUM"))

        # DMA order: x chunk-0 / wqk pairs first so the first qk matmul can
        # start after one pair lands; wv next (v matmuls), wp (proj) last.
        # Startup DMAs split across both HWDGE queues (sync + scalar): the
        # ACT queue is idle until the first exp anyway, and two queues halve
        # the descriptor-issue serialization.
        xt0 = []
        wqk_t = []
        for k in range(KT):
            t = xpool.tile([128, 512], BF16, name=f"xt0_{k}", tag=f"x{k}")
            nc.scalar.dma_start(out=t, in_=xT[k * 128:(k + 1) * 128, 0:512])
            xt0.append(t)
            t = wpool.tile([128, 2 * HG], BF16, name=f"wqk{k}", tag=f"wqk{k}")
            nc.sync.dma_start(out=t, in_=wqk[k * 128:(k + 1) * 128, :])
            wqk_t.append(t)
        tri_sb = consts.tile([128, 128], BF16, name="tri_sb", tag="tri")
        nc.sync.dma_start(out=tri_sb, in_=tri[:, :])
        bqk_sb = []
        for j in range(8):
            t = consts.tile([128, 1], F32, name=f"bqk_sb{j}", tag=f"bqk{j}")
            nc.sync.dma_start(out=t, in_=bqk[j * 128:(j + 1) * 128, :])
            bqk_sb.append(t)
        wv_t = []
        for k in range(KT):
            t = wpool.tile([128, HG], BF16, name=f"wv{k}", tag=f"wv{k}")
            nc.scalar.dma_start(out=t, in_=wv[k * 128:(k + 1) * 128, :])
            wv_t.append(t)
        wp_t = []
        for j in range(4):
            t = wpool.tile([128, N], F32R, name=f"wp{j}", tag=f"wp{j}")
            nc.sync.dma_start(out=t, in_=wp[j * 128:(j + 1) * 128, :])
            wp_t.append(t)

        # residents: qT/kT per head-pair tile [128, T]; v per 128-row chunk,
        # bf16, layout [64 v-cols + ones] x 8 heads = 520 cols
        qT = [resid.tile([128, T], BF16, name=f"qT{j}", tag=f"qT{j}")
              for j in range(4)]
        kTt = [resid.tile([128, T], BF16, name=f"kT{j}", tag=f"kT{j}")
               for j in range(4)]
        vt = [resid.tile([128, 8, 65], BF16, name=f"v{m}", tag=f"v{m}")
              for m in range(NVT)]

        def qkv_units(tck):
            """Filler closures for qkv chunk tck: x DMAs, 8 qk groups,
            4 v groups — emitted one unit at a time between att iters."""
            c0 = tck * 512
            xt = []

            def u_dma():
                for k in range(KT):
                    if tck == 0:
                        xt.append(xt0[k])
                        continue
                    t = xpool.tile([128, 512], BF16, name=f"xt{tck}_{k}",
                                   tag=f"x{k}")
                    nc.sync.dma_start(out=t, in_=xT[k * 128:(k + 1) * 128,
                                                    c0:c0 + 512])
                    xt.append(t)

            def u_qk(jc):
                def f():
                    ps = fillps.tile([128, 512], F32,
                                     name=f"qk_ps{tck}_{jc}", tag="fill")
                    for k in range(KT):
                        nc.tensor.matmul(
                            ps,
                            wqk_t[k][:, jc * 128:(jc + 1) * 128],
                            xt[k],
                            start=(k == 0), stop=(k == KT - 1))
                    dst = (qT[jc] if jc < 4 else kTt[jc - 4])
                    nc.vector.tensor_scalar_add(dst[:, c0:c0 + 512], ps,
                                                bqk_sb[jc])
                return f

            def u_v(mc):
                def f():
                    ps = fillps.tile([128, 512], F32,
                                     name=f"v_ps{tck}_{mc}", tag="fill")
                    for k in range(KT):
                        nc.tensor.matmul(
                            ps,
                            xt[k][:, mc * 128:(mc + 1) * 128],
                            wv_t[k],
                            start=(k == 0), stop=(k == KT - 1))
                    dst = vt[tck * 4 + mc]
                    nc.vector.tensor_copy(
                        dst[:, :, 0:64],
                        ps.rearrange("p (h c) -> p h c", c=64))
                    nc.vector.memset(dst[:, :, 64:65], 1.0)
                return f

            units = [u_dma]
            units += [u_qk(jc) for jc in (0, 4, 1, 5, 2, 6, 3, 7)]
            units += [u_v(mc) for mc in range(4)]
            return units

        def emit_att(qc, filler):
            """Emit attention for q-chunk qc, interleaving the filler unit
            closures evenly across the (hp, kb) iterations."""
            qcol = qc * 512
            nkb = 4 * qc + 4
            n_iters = 4 * nkb
            # unit k fires after iteration floor(k * n_iters / n_units)
            fire_at = {}
            for k in range(len(filler)):
                fire_at.setdefault(k * n_iters // max(1, len(filler)),
                                   []).append(filler[k])
            it = 0
            yts = []
            for hp in range(4):
                y_ps = [ypool.tile([65, 512], F32,
                                   name=f"y_ps{qc}_{hp}_{h01}",
                                   tag=f"y{h01}")
                        for h01 in range(2)]
                for kb in range(nkb):
                    m = kb - 4 * qc          # >= 0 on diagonal blocks
                    lo = m * 128 if m > 0 else 0
                    s_ps = spool.tile([128, 2, 512], F32,
                                      name=f"s{qc}_{hp}_{kb}", tag="s")
                    for h01 in range(2):
                        hb = h01 * 64
                        nc.tensor.matmul(
                            s_ps[:, h01, lo:512],
                            kTt[hp][hb:hb + 64,
                                    kb * 128:(kb + 1) * 128],
                            qT[hp][hb:hb + 64, qcol + lo:qcol + 512],
                            start=True, stop=True)
                    attT = attp.tile([128, 2, 512], BF16,
                                     name=f"attT{qc}_{hp}_{kb}", tag="attT")
                    nc.scalar.activation(out=attT[:, :, lo:512],
                                         in_=s_ps[:, :, lo:512],
                                         func=AF.Exp, scale=0.125)
                    if m >= 0:
                        for h01 in range(2):
                            nc.vector.tensor_mul(attT[:, h01, lo:lo + 128],
                                                 attT[:, h01, lo:lo + 128],
                                                 tri_sb)
                    for h01 in range(2):
                        # av trimmed to the unmasked columns; the masked
                        # prefix was fully written by earlier (lower-kb)
                        # accumulation steps, so group bookkeeping is off.
                        nc.tensor.matmul(
                            y_ps[h01][:, lo:512],
                            vt[kb][:, 2 * hp + h01, :],
                            attT[:, h01, lo:512],
                            start=(kb == 0), stop=(kb == nkb - 1),
                            skip_group_check=True)
                    for f in fire_at.get(it, ()):
                        f()
                    it += 1
                yt = ytp.tile([128, 512], F32R, name=f"yt{qc}_{hp}",
                              tag=f"yt{hp}")
                for h01 in range(2):
                    srow = smallp.tile([1, 512], F32,
                                       name=f"srow{qc}_{hp}_{h01}",
                                       tag=f"srow{h01}")
                    nc.vector.tensor_copy(srow, y_ps[h01][64:65, :])
                    bcst = smallp.tile([64, 512], F32,
                                       name=f"bc{qc}_{hp}_{h01}",
                                       tag=f"bc{h01}")
                    nc.gpsimd.partition_broadcast(bcst, srow)
                    nc.vector.reciprocal_approx_fast(out=bcst, in_=bcst)
                    nc.vector.tensor_mul(
                        yt[h01 * 64:(h01 + 1) * 64, :],
                        y_ps[h01][0:64, :], bcst)
                yts.append(yt)
            return yts

        def proj_units(qc, yts):
            """8 filler closures (one per qb/nch) projecting q-chunk qc."""
            qcol = qc * 512
            o_sb = {}

            def u(qb, nch):
                def f():
                    if nch == 0:
                        o_sb[qb] = outp.tile([128, N], F32,
                                             name=f"o{qc}_{qb}", tag="o")
                    p_ps = fillps.tile([128, 512], F32,
                                       name=f"p{qc}_{qb}_{nch}", tag="fill")
                    for hp in range(4):
                        nc.tensor.matmul(
                            p_ps,
                            yts[hp][:, qb * 128:(qb + 1) * 128],
                            wp_t[hp][:, nch * 512:(nch + 1) * 512],
                            start=(hp == 0), stop=(hp == 3))
                    nc.vector.tensor_copy(
                        o_sb[qb][:, nch * 512:(nch + 1) * 512], p_ps)
                    if nch == 1:
                        nc.sync.dma_start(
                            out=out[qcol + qb * 128:qcol + (qb + 1) * 128,
                                    :],
                            in_=o_sb[qb])
                return f

            return [u(qb, nch) for qb in range(4) for nch in range(2)]

        # qkv(0) runs undiluted up front (ACT has nothing to do yet);
        # qkv(qc+1) spreads through att(qc); the projections of qc 0..2
        # spread through att(3); proj(3) is the tail.
        for u in qkv_units(0):
            u()
        pend_proj = []
        for qc in range(NQC):
            # att(3) spreads the banked projections of qc 0..2, minus four
            # units reserved to cover the PE gap while the final head
            # pair's normalize chain drains before proj(3).
            filler = (qkv_units(qc + 1) if qc < NQC - 1
                      else pend_proj[:-4])
            yts = emit_att(qc, filler)
            pend_proj.extend(proj_units(qc, yts))
        for u in pend_proj[-12:]:
            u()

    nc.compile()
    _CACHE["nc"] = nc
    return nc


def kernel(x, W_attn, b_attn, W_proj, b_proj):
    global _last_results
    nc = _build()

    x = np.asarray(x, dtype=np.float32)
    W_attn = np.asarray(W_attn, dtype=np.float32)
    b_attn = np.asarray(b_attn, dtype=np.float32)
    W_proj = np.asarray(W_proj, dtype=np.float32)
    b_proj = np.asarray(b_proj, dtype=np.float32)

    kk = np.arange(128)[:, None]
    jj = np.arange(128)[None, :]
    tri_np = (jj >= kk).astype(ml_dtypes.bfloat16)

    in_maps = []
    for c in range(NCORES):
        b, hg = divmod(c, 2)
        s = hg * HG
        xT_c = np.ascontiguousarray(x[b].T).astype(ml_dtypes.bfloat16)
        wqk_c = np.ascontiguousarray(
            np.concatenate([W_attn[:, s:s + HG],
                            W_attn[:, N + s:N + s + HG]],
                           axis=1)).astype(ml_dtypes.bfloat16)
        wv_c = np.ascontiguousarray(
            W_attn[:, 2 * N + s:2 * N + s + HG]).astype(ml_dtypes.bfloat16)
        wp_c = np.ascontiguousarray(W_proj[s:s + HG, :])
        bqk_c = np.ascontiguousarray(
            np.concatenate([b_attn[s:s + HG],
                            b_attn[N + s:N + s + HG]]).reshape(2 * HG, 1))
        in_maps.append({
            "xT": xT_c, "wqk": wqk_c, "wv": wv_c, "wp": wp_c,
            "bqk": bqk_c, "tri": tri_np,
        })

    res = run_bass_kernel_spmd(nc, in_maps, list(range(NCORES)))
    _last_results = res
    outs = [res.results[c]["out"] for c in range(NCORES)]
    # v-bias: softmax rows sum to 1, so att @ (xWv + bv) = att @ (xWv) + bv;
    # its projection (bv @ W_proj) plus b_proj are added here, exactly.
    bv = b_attn[2 * N:3 * N]
    extra = bv @ W_proj + b_proj
    y = np.stack([outs[2 * b] + outs[2 * b + 1] for b in range(B)])
    return (y + extra[None, None, :]).astype(np.float32)


# revision 25
# speedup vs baseline: 1.2144x; 1.0271x over previous
"""Multi-head causal attention (B=4, T=2048, N=1024, H=16) on 8 TRN2 NeuronCores.

Sharding: core c = (batch b = c//2, head-group hg = c%2). Each core computes
full-T causal attention for its 8 heads of its batch, plus the partial output
projection for its head rows. Host sums the two head-group partials per batch
and adds b_proj (and the v-bias contribution, which is exact because softmax
rows sum to 1).

Per-core kernel, one globally-scheduled instruction stream (no phase
barriers) so the Tile scheduler can fill ACT-paced attention gaps with
qkv-projection and output-projection matmuls:

  qkv chunk tck (t-cols tck*512):  qkT[j,t] = Wqk.T @ x.T (bias added on
  DVE), v[t,d] = x @ Wv stored bf16 as [64 v | 1] x 8 heads.

  attention (qc, hp): scores TRANSPOSED [k-part, q-free]; the two heads of
  the pair run CONCURRENTLY in the PE array via row-tiling (lhsT at
  partitions 0:64 / 64:128 -> tile_position (0,0)/(64,0)), writing the two
  halves of one [128, 2, 512] PSUM tile.  One fused 1024-wide exp (ACT,
  scale=1/8) covers both heads; diagonal blocks are column-trimmed (the
  fully-masked prefix is neither computed nor exp'd) and only the 128-col
  boundary block is mask-multiplied (bf16 DVE).  The av matmul
  lhsT = v[128,65] (64 v cols + ones) accumulates y.T AND the softmax row
  sums into PSUM [65, 512].  Normalize: DVE reciprocal of the sums row,
  gpsimd partition-broadcast, DVE multiply into the projection-ready yT
  tile.  Projection: yT.T @ Wp in f32r, DMA'd straight from PSUM.

Emission order qkv(0), att(0), qkv(1), proj(0), att(1), ... lets the
dependency scheduler overlap everything; PE stream work (~496k cycles) is
the target critical path.
"""
import numpy as np
import ml_dtypes
from contextlib import ExitStack

import concourse.bass as bass
import concourse.mybir as mybir
from concourse import bacc
from concourse import bass_utils as _bu
from concourse.bass_utils import run_bass_kernel_spmd
from concourse.tile import TileContext


F32 = mybir.dt.float32
F32R = mybir.dt.float32r
BF16 = mybir.dt.bfloat16
AF = mybir.ActivationFunctionType

B, T, N, H = 4, 2048, 1024, 16
Dh = 64
HG = 512            # head-group width per core (8 heads x 64)
NCORES = 8
KT = N // 128       # 8 contraction tiles for qkv
NQC = T // 512      # 4 q-chunks
NVT = T // 128      # 16 v tiles / k blocks

_CACHE = {}
_last_results = None


def _build():
    if "nc" in _CACHE:
        return _CACHE["nc"]

    nc = bacc.Bacc("TRN2", target_bir_lowering=False)

    xT = nc.declare_dram_parameter("xT", [N, T], BF16, isOutput=False)
    wqk = nc.declare_dram_parameter("wqk", [N, 2 * HG], BF16, isOutput=False)
    wv = nc.declare_dram_parameter("wv", [N, HG], BF16, isOutput=False)
    wp = nc.declare_dram_parameter("wp", [HG, N], F32R, isOutput=False)
    bqk = nc.declare_dram_parameter("bqk", [2 * HG, 1], F32, isOutput=False)
    tri = nc.declare_dram_parameter("tri", [128, 128], BF16, isOutput=False)
    out = nc.declare_dram_parameter("out", [T, N], F32, isOutput=True)

    with TileContext(nc) as tc, ExitStack() as st:
        consts = st.enter_context(tc.tile_pool(name="consts", bufs=1))
        wpool = st.enter_context(tc.tile_pool(name="wpool", bufs=1))
        xpool = st.enter_context(tc.tile_pool(name="xpool", bufs=2))
        resid = st.enter_context(tc.tile_pool(name="resid", bufs=1))
        attp = st.enter_context(tc.tile_pool(name="attp", bufs=4))
        smallp = st.enter_context(tc.tile_pool(name="smallp", bufs=2))
        ytp = st.enter_context(tc.tile_pool(name="ytp", bufs=4))
        outp = st.enter_context(tc.tile_pool(name="outp", bufs=2))
        spool = st.enter_context(tc.tile_pool(name="spool", bufs=2,
                                              space="PSUM"))
        ypool = st.enter_context(tc.tile_pool(name="ypool", bufs=1,
                                              space="PSUM"))
        fillps = st.enter_context(tc.tile_pool(name="fillps", bufs=2,
                                               space="PSUM"))

        # DMA order: x chunk-0 / wqk pairs first so the first qk matmul can
        # start after one pair lands; wv next (v matmuls), wp (proj) last.
        # Startup DMAs split across both HWDGE queues (sync + scalar): the
        # ACT queue is idle until the first exp anyway, and two queues halve
        # the descriptor-issue serialization.
        xt0 = []
        wqk_t = []
        for k in range(KT):
            t = xpool.tile([128, 512], BF16, name=f"xt0_{k}", tag=f"x{k}")
            nc.sync.dma_start(out=t, in_=xT[k * 128:(k + 1) * 128, 0:512])
            xt0.append(t)
            t = wpool.tile([128, 2 * HG], BF16, name=f"wqk{k}", tag=f"wqk{k}")
            nc.sync.dma_start(out=t, in_=wqk[k * 128:(k + 1) * 128, :])
            wqk_t.append(t)
        tri_sb = consts.tile([128, 128], BF16, name="tri_sb", tag="tri")
        nc.sync.dma_start(out=tri_sb, in_=tri[:, :])
        bqk_sb = []
        for j in range(8):
            t = consts.tile([128, 1], F32, name=f"bqk_sb{j}", tag=f"bqk{j}")
            nc.sync.dma_start(out=t, in_=bqk[j * 128:(j + 1) * 128, :])
            bqk_sb.append(t)
        wv_t = []
        for k in range(KT):
            t = wpool.tile([128, HG], BF16, name=f"wv{k}", tag=f"wv{k}")
            nc.sync.dma_start(out=t, in_=wv[k * 128:(k + 1) * 128, :])
            wv_t.append(t)
        wp_t = []
        for j in range(4):
            t = wpool.tile([128, N], F32R, name=f"wp{j}", tag=f"wp{j}")
            nc.sync.dma_start(out=t, in_=wp[j * 128:(j + 1) * 128, :])
            wp_t.append(t)

        # residents: qT/kT per head-pair tile [128, T]; v per 128-row chunk,
        # bf16, layout [64 v-cols + ones] x 8 heads = 520 cols
        qT = [resid.tile([128, T], BF16, name=f"qT{j}", tag=f"qT{j}")
              for j in range(4)]
        kTt = [resid.tile([128, T], BF16, name=f"kT{j}", tag=f"kT{j}")
               for j in range(4)]
        vt = [resid.tile([128, 8, 65], BF16, name=f"v{m}", tag=f"v{m}")
              for m in range(NVT)]

        def qkv_units(tck):
            """Filler closures for qkv chunk tck: x DMAs, 8 qk groups,
            4 v groups — emitted one unit at a time between att iters."""
            c0 = tck * 512
            xt = []

            def u_dma():
                for k in range(KT):
                    if tck == 0:
                        xt.append(xt0[k])
                        continue
                    t = xpool.tile([128, 512], BF16, name=f"xt{tck}_{k}",
                                   tag=f"x{k}")
                    nc.sync.dma_start(out=t, in_=xT[k * 128:(k + 1) * 128,
                                                    c0:c0 + 512])
                    xt.append(t)

            def u_qk(jc):
                def f():
                    ps = fillps.tile([128, 512], F32,
                                     name=f"qk_ps{tck}_{jc}", tag="fill")
                    for k in range(KT):
                        nc.tensor.matmul(
                            ps,
                            wqk_t[k][:, jc * 128:(jc + 1) * 128],
                            xt[k],
                            start=(k == 0), stop=(k == KT - 1))
                    dst = (qT[jc] if jc < 4 else kTt[jc - 4])
                    nc.vector.tensor_scalar_add(dst[:, c0:c0 + 512], ps,
                                                bqk_sb[jc])
                return f

            def u_v(mc):
                def f():
                    ps = fillps.tile([128, 512], F32,
                                     name=f"v_ps{tck}_{mc}", tag="fill")
                    for k in range(KT):
                        nc.tensor.matmul(
                            ps,
                            xt[k][:, mc * 128:(mc + 1) * 128],
                            wv_t[k],
                            start=(k == 0), stop=(k == KT - 1))
                    dst = vt[tck * 4 + mc]
                    nc.vector.tensor_copy(
                        dst[:, :, 0:64],
                        ps.rearrange("p (h c) -> p h c", c=64))
                    nc.vector.memset(dst[:, :, 64:65], 1.0)
                return f

            units = [u_dma]
            units += [u_qk(jc) for jc in (0, 4, 1, 5, 2, 6, 3, 7)]
            units += [u_v(mc) for mc in range(4)]
            return units

        def emit_att(qc, filler):
            """Emit attention for q-chunk qc, interleaving the filler unit
            closures evenly across the (hp, kb) iterations."""
            qcol = qc * 512
            nkb = 4 * qc + 4
            n_iters = 4 * nkb
            # unit k fires after iteration floor(k * n_iters / n_units)
            fire_at = {}
            for k in range(len(filler)):
                fire_at.setdefault(k * n_iters // max(1, len(filler)),
                                   []).append(filler[k])
            it = 0
            yts = []
            for hp in range(4):
                y_ps = [ypool.tile([65, 512], F32,
                                   name=f"y_ps{qc}_{hp}_{h01}",
                                   tag=f"y{h01}")
                        for h01 in range(2)]
                for kb in range(nkb):
                    m = kb - 4 * qc          # >= 0 on diagonal blocks
                    lo = m * 128 if m > 0 else 0
                    s_ps = spool.tile([128, 2, 512], F32,
                                      name=f"s{qc}_{hp}_{kb}", tag="s")
                    for h01 in range(2):
                        hb = h01 * 64
                        nc.tensor.matmul(
                            s_ps[:, h01, lo:512],
                            kTt[hp][hb:hb + 64,
                                    kb * 128:(kb + 1) * 128],
                            qT[hp][hb:hb + 64, qcol + lo:qcol + 512],
                            start=True, stop=True)
                    attT = attp.tile([128, 2, 512], BF16,
                                     name=f"attT{qc}_{hp}_{kb}", tag="attT")
                    nc.scalar.activation(out=attT[:, :, lo:512],
                                         in_=s_ps[:, :, lo:512],
                                         func=AF.Exp, scale=0.125)
                    if m >= 0:
                        for h01 in range(2):
                            nc.vector.tensor_mul(attT[:, h01, lo:lo + 128],
                                                 attT[:, h01, lo:lo + 128],
                                                 tri_sb)
                    for h01 in range(2):
                        # av trimmed to the unmasked columns; the masked
                        # prefix was fully written by earlier (lower-kb)
                        # accumulation steps, so group bookkeeping is off.
                        nc.tensor.matmul(
                            y_ps[h01][:, lo:512],
                            vt[kb][:, 2 * hp + h01, :],
                            attT[:, h01, lo:512],
                            start=(kb == 0), stop=(kb == nkb - 1),
                            skip_group_check=True)
                    for f in fire_at.get(it, ()):
                        f()
                    it += 1
                yt = ytp.tile([128, 512], F32R, name=f"yt{qc}_{hp}",
                              tag=f"yt{hp}")
                for h01 in range(2):
                    srow = smallp.tile([1, 512], F32,
                                       name=f"srow{qc}_{hp}_{h01}",
                                       tag=f"srow{h01}")
                    nc.vector.tensor_copy(srow, y_ps[h01][64:65, :])
                    bcst = smallp.tile([64, 512], F32,
                                       name=f"bc{qc}_{hp}_{h01}",
                                       tag=f"bc{h01}")
                    nc.gpsimd.partition_broadcast(bcst, srow)
                    nc.vector.reciprocal_approx_fast(out=bcst, in_=bcst)
                    nc.vector.tensor_mul(
                        yt[h01 * 64:(h01 + 1) * 64, :],
                        y_ps[h01][0:64, :], bcst)
                yts.append(yt)
            return yts

        def proj_units(qc, yts):
            """8 filler closures (one per qb/nch) projecting q-chunk qc."""
            qcol = qc * 512
            o_sb = {}

            def u(qb, nch):
                def f():
                    if nch == 0:
                        o_sb[qb] = outp.tile([128, N], F32,
                                             name=f"o{qc}_{qb}", tag="o")
                    p_ps = fillps.tile([128, 512], F32,
                                       name=f"p{qc}_{qb}_{nch}", tag="fill")
                    for hp in range(4):
                        nc.tensor.matmul(
                            p_ps,
                            yts[hp][:, qb * 128:(qb + 1) * 128],
                            wp_t[hp][:, nch * 512:(nch + 1) * 512],
                            start=(hp == 0), stop=(hp == 3))
                    nc.vector.tensor_copy(
                        o_sb[qb][:, nch * 512:(nch + 1) * 512], p_ps)
                    if nch == 1:
                        nc.sync.dma_start(
                            out=out[qcol + qb * 128:qcol + (qb + 1) * 128,
                                    :],
                            in_=o_sb[qb])
                return f

            return [u(qb, nch) for qb in range(4) for nch in range(2)]

        # qkv(0) runs undiluted up front (ACT has nothing to do yet);
        # qkv(qc+1) spreads through att(qc); the projections of qc 0..2
        # spread through att(3); proj(3) is the tail.
        for u in qkv_units(0):
            u()
        pend_proj = []
        for qc in range(NQC):
            # att(3) spreads the banked projections of qc 0..2, minus four
            # units reserved to cover the PE gap while the final head
            # pair's normalize chain drains before proj(3).
            filler = (qkv_units(qc + 1) if qc < NQC - 1
                      else pend_proj[:-4])
            yts = emit_att(qc, filler)
            pend_proj.extend(proj_units(qc, yts))
        for u in pend_proj[-12:]:
            u()

    nc.compile()
    _CACHE["nc"] = nc
    return nc


def kernel(x, W_attn, b_attn, W_proj, b_proj):
    global _last_results
    nc = _build()

    x = np.asarray(x, dtype=np.float32)
    W_attn = np.asarray(W_attn, dtype=np.float32)
    b_attn = np.asarray(b_attn, dtype=np.float32)
    W_proj = np.asarray(W_proj, dtype=np.float32)
    b_proj = np.asarray(b_proj, dtype=np.float32)

    kk = np.arange(128)[:, None]
    jj = np.arange(128)[None, :]
    tri_np = (jj >= kk).astype(ml_dtypes.bfloat16)

    in_maps = []
    for c in range(NCORES):
        b, hg = divmod(c, 2)
        s = hg * HG
        xT_c = np.ascontiguousarray(x[b].T).astype(ml_dtypes.bfloat16)
        wqk_c = np.ascontiguousarray(
            np.concatenate([W_attn[:, s:s + HG],
                            W_attn[:, N + s:N + s + HG]],
                           axis=1)).astype(ml_dtypes.bfloat16)
        wv_c = np.ascontiguousarray(
            W_attn[:, 2 * N + s:2 * N + s + HG]).astype(ml_dtypes.bfloat16)
        wp_c = np.ascontiguousarray(W_proj[s:s + HG, :])
        bqk_c = np.ascontiguousarray(
            np.concatenate([b_attn[s:s + HG],
                            b_attn[N + s:N + s + HG]]).reshape(2 * HG, 1))
        in_maps.append({
            "xT": xT_c, "wqk": wqk_c, "wv": wv_c, "wp": wp_c,
            "bqk": bqk_c, "tri": tri_np,
        })

    res = run_bass_kernel_spmd(nc, in_maps, list(range(NCORES)))
    _last_results = res
    outs = [res.results[c]["out"] for c in range(NCORES)]
    # v-bias: softmax rows sum to 1, so att @ (xWv + bv) = att @ (xWv) + bv;
    # its projection (bv @ W_proj) plus b_proj are added here, exactly.
    bv = b_attn[2 * N:3 * N]
    extra = bv @ W_proj + b_proj
    y = np.stack([outs[2 * b] + outs[2 * b + 1] for b in range(B)])
    return (y + extra[None, None, :]).astype(np.float32)


# revision 26
# speedup vs baseline: 1.2176x; 1.0026x over previous
"""Multi-head causal attention (B=4, T=2048, N=1024, H=16) on 8 TRN2 NeuronCores.

Sharding: core c = (batch b = c//2, head-group hg = c%2). Each core computes
full-T causal attention for its 8 heads of its batch, plus the partial output
projection for its head rows. Host sums the two head-group partials per batch
and adds b_proj (and the v-bias contribution, which is exact because softmax
rows sum to 1).

Per-core kernel, one globally-scheduled instruction stream (no phase
barriers) so the Tile scheduler can fill ACT-paced attention gaps with
qkv-projection and output-projection matmuls:

  qkv chunk tck (t-cols tck*512):  qkT[j,t] = Wqk.T @ x.T (bias added on
  DVE), v[t,d] = x @ Wv stored bf16 as [64 v | 1] x 8 heads.

  attention (qc, hp): scores TRANSPOSED [k-part, q-free]; the two heads of
  the pair run CONCURRENTLY in the PE array via row-tiling (lhsT at
  partitions 0:64 / 64:128 -> tile_position (0,0)/(64,0)), writing the two
  halves of one [128, 2, 512] PSUM tile.  One fused 1024-wide exp (ACT,
  scale=1/8) covers both heads; diagonal blocks are column-trimmed on the
  scores matmul, the exp AND the av matmul (the fully-masked prefix is
  never computed), with only the 128-col boundary block mask-multiplied
  (bf16 DVE).  The av matmul lhsT = v[128,65] (64 v cols + ones)
  accumulates y.T AND the softmax row sums into PSUM [65, 512].
  Normalize: DVE copy of the sums row to partition 0 (plain-DVE ops handle
  cross-partition APs on HW; custom-DVE ops like reciprocal_approx_fast do
  NOT — that combination silently reads the wrong partition), gpsimd
  partition-broadcast, DVE reciprocal, DVE multiply from PSUM into the
  projection-ready yT tile.  Projection: yT.T @ Wp in f32r, staged to SBUF
  on DVE (DMA cannot read PSUM), then stored.

Scheduling: the Tile list scheduler picks ready instructions by emission
priority, so overlap is engineered by emission order: qkv(0) up front,
then att(qc) with the 13 units of qkv(qc+1) fired evenly between its
(hp, kb) iterations; att(3) instead interleaves the banked projections of
qc 0..2 (minus 4 reserved to cover the final normalize-chain drain), and
proj(3) is the tail.  This keeps the PE dense (no >3.4us idle, so the HAM
clock gate stays at 8/8) through the ACT-paced attention stretches.
"""
import numpy as np
import ml_dtypes
from contextlib import ExitStack

import concourse.bass as bass
import concourse.mybir as mybir
from concourse import bacc
from concourse import bass_utils as _bu
from concourse.bass_utils import run_bass_kernel_spmd
from concourse.tile import TileContext


F32 = mybir.dt.float32
F32R = mybir.dt.float32r
BF16 = mybir.dt.bfloat16
AF = mybir.ActivationFunctionType

B, T, N, H = 4, 2048, 1024, 16
Dh = 64
HG = 512            # head-group width per core (8 heads x 64)
NCORES = 8
KT = N // 128       # 8 contraction tiles for qkv
NQC = T // 512      # 4 q-chunks
NVT = T // 128      # 16 v tiles / k blocks

_CACHE = {}
_last_results = None


def _build():
    if "nc" in _CACHE:
        return _CACHE["nc"]

    nc = bacc.Bacc("TRN2", target_bir_lowering=False)

    xT = nc.declare_dram_parameter("xT", [N, T], BF16, isOutput=False)
    wqk = nc.declare_dram_parameter("wqk", [N, 2 * HG], BF16, isOutput=False)
    wv = nc.declare_dram_parameter("wv", [N, HG], BF16, isOutput=False)
    wp = nc.declare_dram_parameter("wp", [HG, N], F32R, isOutput=False)
    bqk = nc.declare_dram_parameter("bqk", [2 * HG, 1], F32, isOutput=False)
    tri = nc.declare_dram_parameter("tri", [128, 128], BF16, isOutput=False)
    out = nc.declare_dram_parameter("out", [T, N], F32, isOutput=True)

    with TileContext(nc) as tc, ExitStack() as st:
        consts = st.enter_context(tc.tile_pool(name="consts", bufs=1))
        wpool = st.enter_context(tc.tile_pool(name="wpool", bufs=1))
        xpool = st.enter_context(tc.tile_pool(name="xpool", bufs=2))
        resid = st.enter_context(tc.tile_pool(name="resid", bufs=1))
        attp = st.enter_context(tc.tile_pool(name="attp", bufs=4))
        smallp = st.enter_context(tc.tile_pool(name="smallp", bufs=2))
        ytp = st.enter_context(tc.tile_pool(name="ytp", bufs=4))
        outp = st.enter_context(tc.tile_pool(name="outp", bufs=2))
        spool = st.enter_context(tc.tile_pool(name="spool", bufs=2,
                                              space="PSUM"))
        ypool = st.enter_context(tc.tile_pool(name="ypool", bufs=1,
                                              space="PSUM"))
        fillps = st.enter_context(tc.tile_pool(name="fillps", bufs=2,
                                               space="PSUM"))

        # DMA order: x chunk-0 / wqk pairs first so the first qk matmul can
        # start after one pair lands; wv next (v matmuls), wp (proj) last.
        # Startup DMAs split across both HWDGE queues (sync + scalar): the
        # ACT queue is idle until the first exp anyway, and two queues halve
        # the descriptor-issue serialization.
        xt0 = []
        wqk_t = []
        for k in range(KT):
            t = xpool.tile([128, 512], BF16, name=f"xt0_{k}", tag=f"x{k}")
            nc.sync.dma_start(out=t, in_=xT[k * 128:(k + 1) * 128, 0:512])
            xt0.append(t)
            t = wpool.tile([128, 2 * HG], BF16, name=f"wqk{k}", tag=f"wqk{k}")
            nc.sync.dma_start(out=t, in_=wqk[k * 128:(k + 1) * 128, :])
            wqk_t.append(t)
        tri_sb = consts.tile([128, 128], BF16, name="tri_sb", tag="tri")
        nc.sync.dma_start(out=tri_sb, in_=tri[:, :])
        bqk_sb = []
        for j in range(8):
            t = consts.tile([128, 1], F32, name=f"bqk_sb{j}", tag=f"bqk{j}")
            nc.sync.dma_start(out=t, in_=bqk[j * 128:(j + 1) * 128, :])
            bqk_sb.append(t)
        wv_t = []
        for k in range(KT):
            t = wpool.tile([128, HG], BF16, name=f"wv{k}", tag=f"wv{k}")
            nc.sync.dma_start(out=t, in_=wv[k * 128:(k + 1) * 128, :])
            wv_t.append(t)
        wp_t = []
        for j in range(4):
            t = wpool.tile([128, N], F32R, name=f"wp{j}", tag=f"wp{j}")
            nc.sync.dma_start(out=t, in_=wp[j * 128:(j + 1) * 128, :])
            wp_t.append(t)

        # residents: qT/kT per head-pair tile [128, T]; v per 128-row chunk,
        # bf16, layout [64 v-cols + ones] x 8 heads = 520 cols
        qT = [resid.tile([128, T], BF16, name=f"qT{j}", tag=f"qT{j}")
              for j in range(4)]
        kTt = [resid.tile([128, T], BF16, name=f"kT{j}", tag=f"kT{j}")
               for j in range(4)]
        vt = [resid.tile([128, 8, 65], BF16, name=f"v{m}", tag=f"v{m}")
              for m in range(NVT)]

        def qkv_units(tck):
            """Filler closures for qkv chunk tck: x DMAs, 8 qk groups,
            4 v groups — emitted one unit at a time between att iters."""
            c0 = tck * 512
            xt = []

            def u_dma():
                for k in range(KT):
                    if tck == 0:
                        xt.append(xt0[k])
                        continue
                    t = xpool.tile([128, 512], BF16, name=f"xt{tck}_{k}",
                                   tag=f"x{k}")
                    nc.sync.dma_start(out=t, in_=xT[k * 128:(k + 1) * 128,
                                                    c0:c0 + 512])
                    xt.append(t)

            def u_qk(jc):
                def f():
                    ps = fillps.tile([128, 512], F32,
                                     name=f"qk_ps{tck}_{jc}", tag="fill")
                    for k in range(KT):
                        nc.tensor.matmul(
                            ps,
                            wqk_t[k][:, jc * 128:(jc + 1) * 128],
                            xt[k],
                            start=(k == 0), stop=(k == KT - 1))
                    dst = (qT[jc] if jc < 4 else kTt[jc - 4])
                    nc.vector.tensor_scalar_add(dst[:, c0:c0 + 512], ps,
                                                bqk_sb[jc])
                return f

            def u_v(mc):
                def f():
                    ps = fillps.tile([128, 512], F32,
                                     name=f"v_ps{tck}_{mc}", tag="fill")
                    for k in range(KT):
                        nc.tensor.matmul(
                            ps,
                            xt[k][:, mc * 128:(mc + 1) * 128],
                            wv_t[k],
                            start=(k == 0), stop=(k == KT - 1))
                    dst = vt[tck * 4 + mc]
                    nc.vector.tensor_copy(
                        dst[:, :, 0:64],
                        ps.rearrange("p (h c) -> p h c", c=64))
                    nc.vector.memset(dst[:, :, 64:65], 1.0)
                return f

            units = [u_dma]
            units += [u_qk(jc) for jc in (0, 4, 1, 5, 2, 6, 3, 7)]
            units += [u_v(mc) for mc in range(4)]
            return units

        def emit_att(qc, filler):
            """Emit attention for q-chunk qc, interleaving the filler unit
            closures evenly across the (hp, kb) iterations."""
            qcol = qc * 512
            nkb = 4 * qc + 4
            n_iters = 4 * nkb
            # unit k fires after iteration floor(k * n_iters / n_units)
            fire_at = {}
            for k in range(len(filler)):
                fire_at.setdefault(k * n_iters // max(1, len(filler)),
                                   []).append(filler[k])
            it = 0
            yts = []
            for hp in range(4):
                y_ps = [ypool.tile([65, 512], F32,
                                   name=f"y_ps{qc}_{hp}_{h01}",
                                   tag=f"y{h01}")
                        for h01 in range(2)]
                for kb in range(nkb):
                    m = kb - 4 * qc          # >= 0 on diagonal blocks
                    lo = m * 128 if m > 0 else 0
                    s_ps = spool.tile([128, 2, 512], F32,
                                      name=f"s{qc}_{hp}_{kb}", tag="s")
                    for h01 in range(2):
                        hb = h01 * 64
                        nc.tensor.matmul(
                            s_ps[:, h01, lo:512],
                            kTt[hp][hb:hb + 64,
                                    kb * 128:(kb + 1) * 128],
                            qT[hp][hb:hb + 64, qcol + lo:qcol + 512],
                            start=True, stop=True)
                    attT = attp.tile([128, 2, 512], BF16,
                                     name=f"attT{qc}_{hp}_{kb}", tag="attT")
                    nc.scalar.activation(out=attT[:, :, lo:512],
                                         in_=s_ps[:, :, lo:512],
                                         func=AF.Exp, scale=0.125)
                    if m >= 0:
                        for h01 in range(2):
                            nc.vector.tensor_mul(attT[:, h01, lo:lo + 128],
                                                 attT[:, h01, lo:lo + 128],
                                                 tri_sb)
                    for h01 in range(2):
                        # av trimmed to the unmasked columns; the masked
                        # prefix was fully written by earlier (lower-kb)
                        # accumulation steps, so group bookkeeping is off.
                        nc.tensor.matmul(
                            y_ps[h01][:, lo:512],
                            vt[kb][:, 2 * hp + h01, :],
                            attT[:, h01, lo:512],
                            start=(kb == 0), stop=(kb == nkb - 1),
                            skip_group_check=True)
                    for f in fire_at.get(it, ()):
                        f()
                    it += 1
                yt = ytp.tile([128, 512], F32R, name=f"yt{qc}_{hp}",
                              tag=f"yt{hp}")
                for h01 in range(2):
                    srow = smallp.tile([1, 512], F32,
                                       name=f"srow{qc}_{hp}_{h01}",
                                       tag=f"srow{h01}")
                    nc.vector.tensor_copy(srow, y_ps[h01][64:65, :])
                    bcst = smallp.tile([64, 512], F32,
                                       name=f"bc{qc}_{hp}_{h01}",
                                       tag=f"bc{h01}")
                    nc.gpsimd.partition_broadcast(bcst, srow)
                    nc.vector.reciprocal_approx_fast(out=bcst, in_=bcst)
                    nc.vector.tensor_mul(
                        yt[h01 * 64:(h01 + 1) * 64, :],
                        y_ps[h01][0:64, :], bcst)
                yts.append(yt)
            return yts

        def proj_units(qc, yts):
            """8 filler closures (one per qb/nch) projecting q-chunk qc."""
            qcol = qc * 512
            o_sb = {}

            def u(qb, nch):
                def f():
                    if nch == 0:
                        o_sb[qb] = outp.tile([128, N], F32,
                                             name=f"o{qc}_{qb}", tag="o")
                    p_ps = fillps.tile([128, 512], F32,
                                       name=f"p{qc}_{qb}_{nch}", tag="fill")
                    for hp in range(4):
                        nc.tensor.matmul(
                            p_ps,
                            yts[hp][:, qb * 128:(qb + 1) * 128],
                            wp_t[hp][:, nch * 512:(nch + 1) * 512],
                            start=(hp == 0), stop=(hp == 3))
                    nc.vector.tensor_copy(
                        o_sb[qb][:, nch * 512:(nch + 1) * 512], p_ps)
                    if nch == 1:
                        nc.sync.dma_start(
                            out=out[qcol + qb * 128:qcol + (qb + 1) * 128,
                                    :],
                            in_=o_sb[qb])
                return f

            return [u(qb, nch) for qb in range(4) for nch in range(2)]

        # qkv(0) runs undiluted up front (ACT has nothing to do yet);
        # qkv(qc+1) spreads through att(qc); the projections of qc 0..2
        # spread through att(3); proj(3) is the tail.
        for u in qkv_units(0):
            u()
        pend_proj = []
        for qc in range(NQC):
            # att(3) spreads the banked projections of qc 0..2, minus four
            # units reserved to cover the PE gap while the final head
            # pair's normalize chain drains before proj(3).
            filler = (qkv_units(qc + 1) if qc < NQC - 1
                      else pend_proj[:-4])
            yts = emit_att(qc, filler)
            pend_proj.extend(proj_units(qc, yts))
        for u in pend_proj[-12:]:
            u()

    nc.compile()
    _CACHE["nc"] = nc
    return nc


def kernel(x, W_attn, b_attn, W_proj, b_proj):
    global _last_results
    nc = _build()

    x = np.asarray(x, dtype=np.float32)
    W_attn = np.asarray(W_attn, dtype=np.float32)
    b_attn = np.asarray(b_attn, dtype=np.float32)
    W_proj = np.asarray(W_proj, dtype=np.float32)
    b_proj = np.asarray(b_proj, dtype=np.float32)

    kk = np.arange(128)[:, None]
    jj = np.arange(128)[None, :]
    tri_np = (jj >= kk).astype(ml_dtypes.bfloat16)

    in_maps = []
    for c in range(NCORES):
        b, hg = divmod(c, 2)
        s = hg * HG
        xT_c = np.ascontiguousarray(x[b].T).astype(ml_dtypes.bfloat16)
        wqk_c = np.ascontiguousarray(
            np.concatenate([W_attn[:, s:s + HG],
                            W_attn[:, N + s:N + s + HG]],
                           axis=1)).astype(ml_dtypes.bfloat16)
        wv_c = np.ascontiguousarray(
            W_attn[:, 2 * N + s:2 * N + s + HG]).astype(ml_dtypes.bfloat16)
        wp_c = np.ascontiguousarray(W_proj[s:s + HG, :])
        bqk_c = np.ascontiguousarray(
            np.concatenate([b_attn[s:s + HG],
                            b_attn[N + s:N + s + HG]]).reshape(2 * HG, 1))
        in_maps.append({
            "xT": xT_c, "wqk": wqk_c, "wv": wv_c, "wp": wp_c,
            "bqk": bqk_c, "tri": tri_np,
        })

    res = run_bass_kernel_spmd(nc, in_maps, list(range(NCORES)))
    _last_results = res
    outs = [res.results[c]["out"] for c in range(NCORES)]
    # v-bias: softmax rows sum to 1, so att @ (xWv + bv) = att @ (xWv) + bv;
    # its projection (bv @ W_proj) plus b_proj are added here, exactly.
    bv = b_attn[2 * N:3 * N]
    extra = bv @ W_proj + b_proj
    y = np.stack([outs[2 * b] + outs[2 * b + 1] for b in range(B)])
    return (y + extra[None, None, :]).astype(np.float32)


# revision 27
# speedup vs baseline: 1.2542x; 1.0300x over previous
"""Multi-head causal attention (B=4, T=2048, N=1024, H=16) on 8 TRN2 NeuronCores.

Sharding: core c = (batch b = c//2, head-group hg = c%2). Each core computes
full-T causal attention for its 8 heads of its batch, plus the partial output
projection for its head rows. Host sums the two head-group partials per batch
and adds b_proj (and the v-bias contribution, which is exact because softmax
rows sum to 1).

Per-core kernel, one globally-scheduled instruction stream (no phase
barriers) so the Tile scheduler can fill ACT-paced attention gaps with
qkv-projection and output-projection matmuls:

  qkv chunk tck (t-cols tck*512):  qkT[j,t] = Wqk.T @ x.T (bias added on
  DVE), v[t,d] = x @ Wv stored bf16 as [64 v | 1] x 8 heads.

  attention (qc, hp): scores TRANSPOSED [k-part, q-free]; the two heads of
  the pair run CONCURRENTLY in the PE array via row-tiling (lhsT at
  partitions 0:64 / 64:128 -> tile_position (0,0)/(64,0)), writing the two
  halves of one [128, 2, 512] PSUM tile.  One fused 1024-wide exp (ACT,
  scale=1/8) covers both heads; diagonal blocks are column-trimmed on the
  scores matmul, the exp AND the av matmul (the fully-masked prefix is
  never computed), with only the 128-col boundary block mask-multiplied
  (bf16 DVE).  The av matmul lhsT = v[128,65] (64 v cols + ones)
  accumulates y.T AND the softmax row sums into PSUM [65, 512].
  Normalize: DVE copy of the sums row to partition 0 (plain-DVE ops handle
  cross-partition APs on HW; custom-DVE ops like reciprocal_approx_fast do
  NOT — that combination silently reads the wrong partition), gpsimd
  partition-broadcast, DVE reciprocal, DVE multiply from PSUM into the
  projection-ready yT tile.  Projection: yT.T @ Wp in f32r, staged to SBUF
  on DVE (DMA cannot read PSUM), then stored.

Scheduling: the Tile list scheduler picks ready instructions by emission
priority, so overlap is engineered by emission order: qkv(0) up front,
then att(qc) with the 13 units of qkv(qc+1) fired evenly between its
(hp, kb) iterations; att(3) instead interleaves the banked projections of
qc 0..2 (minus 4 reserved to cover the final normalize-chain drain), and
proj(3) is the tail.  This keeps the PE dense (no >3.4us idle, so the HAM
clock gate stays at 8/8) through the ACT-paced attention stretches.
"""
import numpy as np
import ml_dtypes
from contextlib import ExitStack

import concourse.bass as bass
import concourse.mybir as mybir
from concourse import bacc
from concourse import bass_utils as _bu
from concourse.bass_utils import run_bass_kernel_spmd
from concourse.tile import TileContext


F32 = mybir.dt.float32
F32R = mybir.dt.float32r
BF16 = mybir.dt.bfloat16
AF = mybir.ActivationFunctionType

B, T, N, H = 4, 2048, 1024, 16
Dh = 64
HG = 512            # head-group width per core (8 heads x 64)
NCORES = 8
KT = N // 128       # 8 contraction tiles for qkv
NQC = T // 512      # 4 q-chunks
NVT = T // 128      # 16 v tiles / k blocks

_CACHE = {}
_last_results = None


def _build():
    if "nc" in _CACHE:
        return _CACHE["nc"]

    nc = bacc.Bacc("TRN2", target_bir_lowering=False)

    xT = nc.declare_dram_parameter("xT", [N, T], BF16, isOutput=False)
    wqk = nc.declare_dram_parameter("wqk", [N, 2 * HG], BF16, isOutput=False)
    wv = nc.declare_dram_parameter("wv", [N, HG], BF16, isOutput=False)
    wp = nc.declare_dram_parameter("wp", [HG, N], F32R, isOutput=False)
    bqk = nc.declare_dram_parameter("bqk", [2 * HG, 1], F32, isOutput=False)
    tri = nc.declare_dram_parameter("tri", [128, 128], BF16, isOutput=False)
    out = nc.declare_dram_parameter("out", [T, N], F32, isOutput=True)

    with TileContext(nc) as tc, ExitStack() as st:
        consts = st.enter_context(tc.tile_pool(name="consts", bufs=1))
        wpool = st.enter_context(tc.tile_pool(name="wpool", bufs=1))
        xpool = st.enter_context(tc.tile_pool(name="xpool", bufs=2))
        resid = st.enter_context(tc.tile_pool(name="resid", bufs=1))
        attp = st.enter_context(tc.tile_pool(name="attp", bufs=4))
        smallp = st.enter_context(tc.tile_pool(name="smallp", bufs=2))
        ytp = st.enter_context(tc.tile_pool(name="ytp", bufs=4))
        outp = st.enter_context(tc.tile_pool(name="outp", bufs=2))
        spool = st.enter_context(tc.tile_pool(name="spool", bufs=2,
                                              space="PSUM"))
        ypool = st.enter_context(tc.tile_pool(name="ypool", bufs=1,
                                              space="PSUM"))
        fillps = st.enter_context(tc.tile_pool(name="fillps", bufs=2,
                                               space="PSUM"))

        # DMA order: x chunk-0 / wqk pairs first so the first qk matmul can
        # start after one pair lands; wv next (v matmuls), wp (proj) last.
        # Startup DMAs split across both HWDGE queues (sync + scalar): the
        # ACT queue is idle until the first exp anyway, and two queues halve
        # the descriptor-issue serialization.
        xt0 = []
        wqk_t = []
        for k in range(KT):
            t = xpool.tile([128, 512], BF16, name=f"xt0_{k}", tag=f"x{k}")
            nc.sync.dma_start(out=t, in_=xT[k * 128:(k + 1) * 128, 0:512])
            xt0.append(t)
            t = wpool.tile([128, 2 * HG], BF16, name=f"wqk{k}", tag=f"wqk{k}")
            nc.sync.dma_start(out=t, in_=wqk[k * 128:(k + 1) * 128, :])
            wqk_t.append(t)
        tri_sb = consts.tile([128, 128], BF16, name="tri_sb", tag="tri")
        nc.sync.dma_start(out=tri_sb, in_=tri[:, :])
        bqk_sb = []
        for j in range(8):
            t = consts.tile([128, 1], F32, name=f"bqk_sb{j}", tag=f"bqk{j}")
            nc.sync.dma_start(out=t, in_=bqk[j * 128:(j + 1) * 128, :])
            bqk_sb.append(t)
        wv_t = []
        for k in range(KT):
            t = wpool.tile([128, HG], BF16, name=f"wv{k}", tag=f"wv{k}")
            nc.sync.dma_start(out=t, in_=wv[k * 128:(k + 1) * 128, :])
            wv_t.append(t)
        wp_t = []
        for j in range(4):
            t = wpool.tile([128, N], F32R, name=f"wp{j}", tag=f"wp{j}")
            nc.sync.dma_start(out=t, in_=wp[j * 128:(j + 1) * 128, :])
            wp_t.append(t)

        # residents: qT/kT per head-pair tile [128, T]; v per 128-row chunk,
        # bf16, layout [64 v-cols + ones] x 8 heads = 520 cols
        qT = [resid.tile([128, T], BF16, name=f"qT{j}", tag=f"qT{j}")
              for j in range(4)]
        kTt = [resid.tile([128, T], BF16, name=f"kT{j}", tag=f"kT{j}")
               for j in range(4)]
        vt = [resid.tile([128, 8, 65], BF16, name=f"v{m}", tag=f"v{m}")
              for m in range(NVT)]

        def qkv_units(tck):
            """Filler closures for qkv chunk tck: x DMAs, 8 qk groups,
            4 v groups — emitted one unit at a time between att iters."""
            c0 = tck * 512
            xt = []

            def u_dma():
                for k in range(KT):
                    if tck == 0:
                        xt.append(xt0[k])
                        continue
                    t = xpool.tile([128, 512], BF16, name=f"xt{tck}_{k}",
                                   tag=f"x{k}")
                    nc.sync.dma_start(out=t, in_=xT[k * 128:(k + 1) * 128,
                                                    c0:c0 + 512])
                    xt.append(t)

            def u_qk(jc):
                def f():
                    ps = fillps.tile([128, 512], F32,
                                     name=f"qk_ps{tck}_{jc}", tag="fill")
                    for k in range(KT):
                        nc.tensor.matmul(
                            ps,
                            wqk_t[k][:, jc * 128:(jc + 1) * 128],
                            xt[k],
                            start=(k == 0), stop=(k == KT - 1))
                    dst = (qT[jc] if jc < 4 else kTt[jc - 4])
                    nc.vector.tensor_scalar_add(dst[:, c0:c0 + 512], ps,
                                                bqk_sb[jc])
                return f

            def u_v(mc):
                def f():
                    ps = fillps.tile([128, 512], F32,
                                     name=f"v_ps{tck}_{mc}", tag="fill")
                    for k in range(KT):
                        nc.tensor.matmul(
                            ps,
                            xt[k][:, mc * 128:(mc + 1) * 128],
                            wv_t[k],
                            start=(k == 0), stop=(k == KT - 1))
                    dst = vt[tck * 4 + mc]
                    nc.vector.tensor_copy(
                        dst[:, :, 0:64],
                        ps.rearrange("p (h c) -> p h c", c=64))
                    nc.vector.memset(dst[:, :, 64:65], 1.0)
                return f

            units = [u_dma]
            units += [u_qk(jc) for jc in (0, 4, 1, 5, 2, 6, 3, 7)]
            units += [u_v(mc) for mc in range(4)]
            return units

        def emit_att(qc, filler):
            """Emit attention for q-chunk qc, interleaving the filler unit
            closures evenly across the (hp, kb) iterations."""
            qcol = qc * 512
            nkb = 4 * qc + 4
            n_iters = 4 * nkb
            # unit k fires after iteration floor(k * n_iters / n_units)
            fire_at = {}
            for k in range(len(filler)):
                fire_at.setdefault(k * n_iters // max(1, len(filler)),
                                   []).append(filler[k])
            it = 0
            yts = []
            for hp in range(4):
                y_ps = [ypool.tile([65, 512], F32,
                                   name=f"y_ps{qc}_{hp}_{h01}",
                                   tag=f"y{h01}")
                        for h01 in range(2)]
                for kb in range(nkb):
                    m = kb - 4 * qc          # >= 0 on diagonal blocks
                    lo = m * 128 if m > 0 else 0
                    s_ps = spool.tile([128, 2, 512], F32,
                                      name=f"s{qc}_{hp}_{kb}", tag="s")
                    for h01 in range(2):
                        hb = h01 * 64
                        nc.tensor.matmul(
                            s_ps[:, h01, lo:512],
                            kTt[hp][hb:hb + 64,
                                    kb * 128:(kb + 1) * 128],
                            qT[hp][hb:hb + 64, qcol + lo:qcol + 512],
                            start=True, stop=True)
                    attT = attp.tile([128, 2, 512], BF16,
                                     name=f"attT{qc}_{hp}_{kb}", tag="attT")
                    nc.scalar.activation(out=attT[:, :, lo:512],
                                         in_=s_ps[:, :, lo:512],
                                         func=AF.Exp, scale=0.125)
                    if m >= 0:
                        for h01 in range(2):
                            nc.vector.tensor_mul(attT[:, h01, lo:lo + 128],
                                                 attT[:, h01, lo:lo + 128],
                                                 tri_sb)
                    for h01 in range(2):
                        # av trimmed to the unmasked columns; the masked
                        # prefix was fully written by earlier (lower-kb)
                        # accumulation steps, so group bookkeeping is off.
                        nc.tensor.matmul(
                            y_ps[h01][:, lo:512],
                            vt[kb][:, 2 * hp + h01, :],
                            attT[:, h01, lo:512],
                            start=(kb == 0), stop=(kb == nkb - 1),
                            skip_group_check=True)
                    for f in fire_at.get(it, ()):
                        f()
                    it += 1
                yt = ytp.tile([128, 512], F32R, name=f"yt{qc}_{hp}",
                              tag=f"yt{hp}")
                for h01 in range(2):
                    # Copy sums + y out of PSUM first: these two copies are
                    # the only y_ps readers, so the bank frees after ~0.9us
                    # and the next head pair's AV can start while the
                    # broadcast/reciprocal/mul chain drains off-path.
                    srow = smallp.tile([1, 512], F32,
                                       name=f"srow{qc}_{hp}_{h01}",
                                       tag=f"srow{h01}")
                    nc.vector.tensor_copy(srow, y_ps[h01][64:65, :])
                    ystg = smallp.tile([64, 512], F32,
                                       name=f"ystg{qc}_{hp}_{h01}",
                                       tag=f"ystg{h01}")
                    nc.vector.tensor_copy(ystg, y_ps[h01][0:64, :])
                    bcst = smallp.tile([64, 512], F32,
                                       name=f"bc{qc}_{hp}_{h01}",
                                       tag=f"bc{h01}")
                    nc.gpsimd.partition_broadcast(bcst, srow)
                    nc.vector.reciprocal_approx_fast(out=bcst, in_=bcst)
                    nc.vector.tensor_mul(
                        yt[h01 * 64:(h01 + 1) * 64, :], ystg, bcst)
                yts.append(yt)
            return yts

        def proj_units(qc, yts):
            """8 filler closures (one per qb/nch) projecting q-chunk qc."""
            qcol = qc * 512
            o_sb = {}

            def u(qb, nch):
                def f():
                    if nch == 0:
                        o_sb[qb] = outp.tile([128, N], F32,
                                             name=f"o{qc}_{qb}", tag="o")
                    p_ps = fillps.tile([128, 512], F32,
                                       name=f"p{qc}_{qb}_{nch}", tag="fill")
                    for hp in range(4):
                        nc.tensor.matmul(
                            p_ps,
                            yts[hp][:, qb * 128:(qb + 1) * 128],
                            wp_t[hp][:, nch * 512:(nch + 1) * 512],
                            start=(hp == 0), stop=(hp == 3))
                    nc.vector.tensor_copy(
                        o_sb[qb][:, nch * 512:(nch + 1) * 512], p_ps)
                    if nch == 1:
                        nc.sync.dma_start(
                            out=out[qcol + qb * 128:qcol + (qb + 1) * 128,
                                    :],
                            in_=o_sb[qb])
                return f

            return [u(qb, nch) for qb in range(4) for nch in range(2)]

        # qkv(0) runs undiluted up front (ACT has nothing to do yet);
        # qkv(qc+1) spreads through att(qc); the projections of qc 0..2
        # spread through att(3); proj(3) is the tail.
        for u in qkv_units(0):
            u()
        pend_proj = []
        for qc in range(NQC):
            # att(3) spreads the banked projections of qc 0..2, minus four
            # units reserved to cover the PE gap while the final head
            # pair's normalize chain drains before proj(3).
            filler = (qkv_units(qc + 1) if qc < NQC - 1
                      else pend_proj[:-4])
            yts = emit_att(qc, filler)
            pend_proj.extend(proj_units(qc, yts))
        for u in pend_proj[-12:]:
            u()

    nc.compile()
    _CACHE["nc"] = nc
    return nc


def kernel(x, W_attn, b_attn, W_proj, b_proj):
    global _last_results
    nc = _build()

    x = np.asarray(x, dtype=np.float32)
    W_attn = np.asarray(W_attn, dtype=np.float32)
    b_attn = np.asarray(b_attn, dtype=np.float32)
    W_proj = np.asarray(W_proj, dtype=np.float32)
    b_proj = np.asarray(b_proj, dtype=np.float32)

    kk = np.arange(128)[:, None]
    jj = np.arange(128)[None, :]
    tri_np = (jj >= kk).astype(ml_dtypes.bfloat16)

    in_maps = []
    for c in range(NCORES):
        b, hg = divmod(c, 2)
        s = hg * HG
        xT_c = np.ascontiguousarray(x[b].T).astype(ml_dtypes.bfloat16)
        wqk_c = np.ascontiguousarray(
            np.concatenate([W_attn[:, s:s + HG],
                            W_attn[:, N + s:N + s + HG]],
                           axis=1)).astype(ml_dtypes.bfloat16)
        wv_c = np.ascontiguousarray(
            W_attn[:, 2 * N + s:2 * N + s + HG]).astype(ml_dtypes.bfloat16)
        wp_c = np.ascontiguousarray(W_proj[s:s + HG, :])
        bqk_c = np.ascontiguousarray(
            np.concatenate([b_attn[s:s + HG],
                            b_attn[N + s:N + s + HG]]).reshape(2 * HG, 1))
        in_maps.append({
            "xT": xT_c, "wqk": wqk_c, "wv": wv_c, "wp": wp_c,
            "bqk": bqk_c, "tri": tri_np,
        })

    res = run_bass_kernel_spmd(nc, in_maps, list(range(NCORES)))
    _last_results = res
    outs = [res.results[c]["out"] for c in range(NCORES)]
    # v-bias: softmax rows sum to 1, so att @ (xWv + bv) = att @ (xWv) + bv;
    # its projection (bv @ W_proj) plus b_proj are added here, exactly.
    bv = b_attn[2 * N:3 * N]
    extra = bv @ W_proj + b_proj
    y = np.stack([outs[2 * b] + outs[2 * b + 1] for b in range(B)])
    return (y + extra[None, None, :]).astype(np.float32)


# revision 28
# speedup vs baseline: 1.2683x; 1.0113x over previous
"""Multi-head causal attention (B=4, T=2048, N=1024, H=16) on 8 TRN2 NeuronCores.

Sharding: core c = (batch b = c//2, head-group hg = c%2). Each core computes
full-T causal attention for its 8 heads of its batch, plus the partial output
projection for its head rows. Host sums the two head-group partials per batch
and adds b_proj (and the v-bias contribution, which is exact because softmax
rows sum to 1).

Per-core kernel, one globally-scheduled instruction stream (no phase
barriers) so the Tile scheduler can fill ACT-paced attention gaps with
qkv-projection and output-projection matmuls:

  qkv chunk tck (t-cols tck*512):  qkT[j,t] = Wqk.T @ x.T (bias added on
  DVE), v[t,d] = x @ Wv stored bf16 as [64 v | 1] x 8 heads.

  attention (qc, hp): scores TRANSPOSED [k-part, q-free]; the two heads of
  the pair run CONCURRENTLY in the PE array via row-tiling (lhsT at
  partitions 0:64 / 64:128 -> tile_position (0,0)/(64,0)), writing the two
  halves of one [128, 2, 512] PSUM tile.  One fused 1024-wide exp (ACT,
  scale=1/8) covers both heads; diagonal blocks are column-trimmed on the
  scores matmul, the exp AND the av matmul (the fully-masked prefix is
  never computed), with only the 128-col boundary block mask-multiplied
  (bf16 DVE).  The av matmul lhsT = v[128,65] (64 v cols + ones)
  accumulates y.T AND the softmax row sums into PSUM [65, 512].
  Normalize: DVE copy of the sums row to partition 0 (plain-DVE ops handle
  cross-partition APs on HW; custom-DVE ops like reciprocal_approx_fast do
  NOT — that combination silently reads the wrong partition), gpsimd
  partition-broadcast, DVE reciprocal, DVE multiply from PSUM into the
  projection-ready yT tile.  Projection: yT.T @ Wp in f32r, staged to SBUF
  on DVE (DMA cannot read PSUM), then stored.

Scheduling: the Tile list scheduler picks ready instructions by emission
priority, so overlap is engineered by emission order: qkv(0) up front,
then att(qc) with the 13 units of qkv(qc+1) fired evenly between its
(hp, kb) iterations; att(3) instead interleaves the banked projections of
qc 0..2 (minus 4 reserved to cover the final normalize-chain drain), and
proj(3) is the tail.  This keeps the PE dense (no >3.4us idle, so the HAM
clock gate stays at 8/8) through the ACT-paced attention stretches.
"""
import numpy as np
import ml_dtypes
from contextlib import ExitStack

import concourse.bass as bass
import concourse.mybir as mybir
from concourse import bacc
from concourse import bass_utils as _bu
from concourse.bass_utils import run_bass_kernel_spmd
from concourse.tile import TileContext


F32 = mybir.dt.float32
F32R = mybir.dt.float32r
BF16 = mybir.dt.bfloat16
AF = mybir.ActivationFunctionType

B, T, N, H = 4, 2048, 1024, 16
Dh = 64
HG = 512            # head-group width per core (8 heads x 64)
NCORES = 8
KT = N // 128       # 8 contraction tiles for qkv
NQC = T // 512      # 4 q-chunks
NVT = T // 128      # 16 v tiles / k blocks

_CACHE = {}
_last_results = None


def _build():
    if "nc" in _CACHE:
        return _CACHE["nc"]

    nc = bacc.Bacc("TRN2", target_bir_lowering=False)

    xT = nc.declare_dram_parameter("xT", [N, T], BF16, isOutput=False)
    wqk = nc.declare_dram_parameter("wqk", [N, 2 * HG], BF16, isOutput=False)
    wv = nc.declare_dram_parameter("wv", [N, HG], BF16, isOutput=False)
    wp = nc.declare_dram_parameter("wp", [HG, N], F32R, isOutput=False)
    bqk = nc.declare_dram_parameter("bqk", [2 * HG, 1], F32, isOutput=False)
    tri = nc.declare_dram_parameter("tri", [128, 128], BF16, isOutput=False)
    out = nc.declare_dram_parameter("out", [T, N], F32, isOutput=True)

    with TileContext(nc) as tc, ExitStack() as st:
        consts = st.enter_context(tc.tile_pool(name="consts", bufs=1))
        wpool = st.enter_context(tc.tile_pool(name="wpool", bufs=1))
        xpool = st.enter_context(tc.tile_pool(name="xpool", bufs=2))
        resid = st.enter_context(tc.tile_pool(name="resid", bufs=1))
        attp = st.enter_context(tc.tile_pool(name="attp", bufs=4))
        smallp = st.enter_context(tc.tile_pool(name="smallp", bufs=2))
        ytp = st.enter_context(tc.tile_pool(name="ytp", bufs=4))
        outp = st.enter_context(tc.tile_pool(name="outp", bufs=3))
        spool = st.enter_context(tc.tile_pool(name="spool", bufs=2,
                                              space="PSUM"))
        ypool = st.enter_context(tc.tile_pool(name="ypool", bufs=1,
                                              space="PSUM"))
        fillps = st.enter_context(tc.tile_pool(name="fillps", bufs=2,
                                               space="PSUM"))

        # DMA order: x chunk-0 / wqk pairs first so the first qk matmul can
        # start after one pair lands; wv next (v matmuls), wp (proj) last.
        # Startup DMAs split across both HWDGE queues (sync + scalar): the
        # ACT queue is idle until the first exp anyway, and two queues halve
        # the descriptor-issue serialization.
        xt0 = []
        wqk_t = []
        for k in range(KT):
            t = xpool.tile([128, 512], BF16, name=f"xt0_{k}", tag=f"x{k}")
            nc.sync.dma_start(out=t, in_=xT[k * 128:(k + 1) * 128, 0:512])
            xt0.append(t)
            t = wpool.tile([128, 2 * HG], BF16, name=f"wqk{k}", tag=f"wqk{k}")
            nc.sync.dma_start(out=t, in_=wqk[k * 128:(k + 1) * 128, :])
            wqk_t.append(t)
        tri_sb = consts.tile([128, 128], BF16, name="tri_sb", tag="tri")
        nc.sync.dma_start(out=tri_sb, in_=tri[:, :])
        bqk_sb = []
        for j in range(8):
            t = consts.tile([128, 1], F32, name=f"bqk_sb{j}", tag=f"bqk{j}")
            nc.sync.dma_start(out=t, in_=bqk[j * 128:(j + 1) * 128, :])
            bqk_sb.append(t)
        wv_t = []
        for k in range(KT):
            t = wpool.tile([128, HG], BF16, name=f"wv{k}", tag=f"wv{k}")
            nc.sync.dma_start(out=t, in_=wv[k * 128:(k + 1) * 128, :])
            wv_t.append(t)
        wp_t = []
        for j in range(4):
            t = wpool.tile([128, N], F32R, name=f"wp{j}", tag=f"wp{j}")
            nc.sync.dma_start(out=t, in_=wp[j * 128:(j + 1) * 128, :])
            wp_t.append(t)

        # residents: qT/kT per head-pair tile [128, T]; v per 128-row chunk,
        # bf16, layout [64 v-cols + ones] x 8 heads = 520 cols
        qT = [resid.tile([128, T], BF16, name=f"qT{j}", tag=f"qT{j}")
              for j in range(4)]
        kTt = [resid.tile([128, T], BF16, name=f"kT{j}", tag=f"kT{j}")
               for j in range(4)]
        vt = [resid.tile([128, 8, 65], BF16, name=f"v{m}", tag=f"v{m}")
              for m in range(NVT)]

        def qkv_units(tck):
            """Filler closures for qkv chunk tck: x DMAs, 8 qk groups,
            4 v groups — emitted one unit at a time between att iters."""
            c0 = tck * 512
            xt = []

            def u_dma():
                for k in range(KT):
                    if tck == 0:
                        xt.append(xt0[k])
                        continue
                    t = xpool.tile([128, 512], BF16, name=f"xt{tck}_{k}",
                                   tag=f"x{k}")
                    nc.sync.dma_start(out=t, in_=xT[k * 128:(k + 1) * 128,
                                                    c0:c0 + 512])
                    xt.append(t)

            def u_qk(jc):
                def f():
                    ps = fillps.tile([128, 512], F32,
                                     name=f"qk_ps{tck}_{jc}", tag="fill")
                    for k in range(KT):
                        nc.tensor.matmul(
                            ps,
                            wqk_t[k][:, jc * 128:(jc + 1) * 128],
                            xt[k],
                            start=(k == 0), stop=(k == KT - 1))
                    dst = (qT[jc] if jc < 4 else kTt[jc - 4])
                    nc.vector.tensor_scalar_add(dst[:, c0:c0 + 512], ps,
                                                bqk_sb[jc])
                return f

            def u_v(mc):
                def f():
                    ps = fillps.tile([128, 512], F32,
                                     name=f"v_ps{tck}_{mc}", tag="fill")
                    for k in range(KT):
                        nc.tensor.matmul(
                            ps,
                            xt[k][:, mc * 128:(mc + 1) * 128],
                            wv_t[k],
                            start=(k == 0), stop=(k == KT - 1))
                    dst = vt[tck * 4 + mc]
                    nc.vector.tensor_copy(
                        dst[:, :, 0:64],
                        ps.rearrange("p (h c) -> p h c", c=64))
                    nc.vector.memset(dst[:, :, 64:65], 1.0)
                return f

            units = [u_dma]
            units += [u_qk(jc) for jc in (0, 4, 1, 5, 2, 6, 3, 7)]
            units += [u_v(mc) for mc in range(4)]
            return units

        def emit_att(qc, filler):
            """Emit attention for q-chunk qc, interleaving the filler unit
            closures evenly across the (hp, kb) iterations."""
            qcol = qc * 512
            nkb = 4 * qc + 4
            n_iters = 4 * nkb
            # unit k fires after iteration floor(k * n_iters / n_units)
            fire_at = {}
            for k in range(len(filler)):
                fire_at.setdefault(k * n_iters // max(1, len(filler)),
                                   []).append(filler[k])
            it = 0
            yts = []
            for hp in range(4):
                y_ps = [ypool.tile([65, 512], F32,
                                   name=f"y_ps{qc}_{hp}_{h01}",
                                   tag=f"y{h01}")
                        for h01 in range(2)]
                for kb in range(nkb):
                    m = kb - 4 * qc          # >= 0 on diagonal blocks
                    lo = m * 128 if m > 0 else 0
                    s_ps = spool.tile([128, 2, 512], F32,
                                      name=f"s{qc}_{hp}_{kb}", tag="s")
                    for h01 in range(2):
                        hb = h01 * 64
                        nc.tensor.matmul(
                            s_ps[:, h01, lo:512],
                            kTt[hp][hb:hb + 64,
                                    kb * 128:(kb + 1) * 128],
                            qT[hp][hb:hb + 64, qcol + lo:qcol + 512],
                            start=True, stop=True)
                    attT = attp.tile([128, 2, 512], BF16,
                                     name=f"attT{qc}_{hp}_{kb}", tag="attT")
                    nc.scalar.activation(out=attT[:, :, lo:512],
                                         in_=s_ps[:, :, lo:512],
                                         func=AF.Exp, scale=0.125)
                    if m >= 0:
                        for h01 in range(2):
                            nc.vector.tensor_mul(attT[:, h01, lo:lo + 128],
                                                 attT[:, h01, lo:lo + 128],
                                                 tri_sb)
                    for h01 in range(2):
                        # av trimmed to the unmasked columns; the masked
                        # prefix was fully written by earlier (lower-kb)
                        # accumulation steps, so group bookkeeping is off.
                        nc.tensor.matmul(
                            y_ps[h01][:, lo:512],
                            vt[kb][:, 2 * hp + h01, :],
                            attT[:, h01, lo:512],
                            start=(kb == 0), stop=(kb == nkb - 1),
                            skip_group_check=True)
                    for f in fire_at.get(it, ()):
                        f()
                    it += 1
                yt = ytp.tile([128, 512], F32R, name=f"yt{qc}_{hp}",
                              tag=f"yt{hp}")
                for h01 in range(2):
                    # Copy sums + y out of PSUM first: these two copies are
                    # the only y_ps readers, so the bank frees after ~0.9us
                    # and the next head pair's AV can start while the
                    # broadcast/reciprocal/mul chain drains off-path.
                    srow = smallp.tile([1, 512], F32,
                                       name=f"srow{qc}_{hp}_{h01}",
                                       tag=f"srow{h01}")
                    nc.vector.tensor_copy(srow, y_ps[h01][64:65, :])
                    ystg = smallp.tile([64, 512], F32,
                                       name=f"ystg{qc}_{hp}_{h01}",
                                       tag=f"ystg{h01}")
                    nc.vector.tensor_copy(ystg, y_ps[h01][0:64, :])
                    bcst = smallp.tile([64, 512], F32,
                                       name=f"bc{qc}_{hp}_{h01}",
                                       tag=f"bc{h01}")
                    nc.gpsimd.partition_broadcast(bcst, srow)
                    nc.vector.reciprocal_approx_fast(out=bcst, in_=bcst)
                    nc.vector.tensor_mul(
                        yt[h01 * 64:(h01 + 1) * 64, :], ystg, bcst)
                yts.append(yt)
            return yts

        def proj_units(qc, yts):
            """8 filler closures (one per qb/nch) projecting q-chunk qc."""
            qcol = qc * 512
            o_sb = {}

            def u(qb, nch):
                def f():
                    if nch == 0:
                        o_sb[qb] = outp.tile([128, N], F32,
                                             name=f"o{qc}_{qb}", tag="o")
                    p_ps = fillps.tile([128, 512], F32,
                                       name=f"p{qc}_{qb}_{nch}", tag="fill")
                    for hp in range(4):
                        nc.tensor.matmul(
                            p_ps,
                            yts[hp][:, qb * 128:(qb + 1) * 128],
                            wp_t[hp][:, nch * 512:(nch + 1) * 512],
                            start=(hp == 0), stop=(hp == 3))
                    nc.vector.tensor_copy(
                        o_sb[qb][:, nch * 512:(nch + 1) * 512], p_ps)
                    nc.sync.dma_start(
                        out=out[qcol + qb * 128:qcol + (qb + 1) * 128,
                                nch * 512:(nch + 1) * 512],
                        in_=o_sb[qb][:, nch * 512:(nch + 1) * 512])
                return f

            return [u(qb, nch) for qb in range(4) for nch in range(2)]

        # qkv(0) runs undiluted up front (ACT has nothing to do yet);
        # qkv(qc+1) spreads through att(qc); the projections of qc 0..2
        # spread through att(3); proj(3) is the tail.
        for u in qkv_units(0):
            u()
        pend_proj = []
        for qc in range(NQC):
            # att(3) spreads the banked projections of qc 0..2, minus four
            # units reserved to cover the PE gap while the final head
            # pair's normalize chain drains before proj(3).
            filler = (qkv_units(qc + 1) if qc < NQC - 1
                      else pend_proj[:-4])
            yts = emit_att(qc, filler)
            pend_proj.extend(proj_units(qc, yts))
        for u in pend_proj[-12:]:
            u()

    nc.compile()
    _CACHE["nc"] = nc
    return nc


def kernel(x, W_attn, b_attn, W_proj, b_proj):
    global _last_results
    nc = _build()

    x = np.asarray(x, dtype=np.float32)
    W_attn = np.asarray(W_attn, dtype=np.float32)
    b_attn = np.asarray(b_attn, dtype=np.float32)
    W_proj = np.asarray(W_proj, dtype=np.float32)
    b_proj = np.asarray(b_proj, dtype=np.float32)

    kk = np.arange(128)[:, None]
    jj = np.arange(128)[None, :]
    tri_np = (jj >= kk).astype(ml_dtypes.bfloat16)

    in_maps = []
    for c in range(NCORES):
        b, hg = divmod(c, 2)
        s = hg * HG
        xT_c = np.ascontiguousarray(x[b].T).astype(ml_dtypes.bfloat16)
        wqk_c = np.ascontiguousarray(
            np.concatenate([W_attn[:, s:s + HG],
                            W_attn[:, N + s:N + s + HG]],
                           axis=1)).astype(ml_dtypes.bfloat16)
        wv_c = np.ascontiguousarray(
            W_attn[:, 2 * N + s:2 * N + s + HG]).astype(ml_dtypes.bfloat16)
        wp_c = np.ascontiguousarray(W_proj[s:s + HG, :])
        bqk_c = np.ascontiguousarray(
            np.concatenate([b_attn[s:s + HG],
                            b_attn[N + s:N + s + HG]]).reshape(2 * HG, 1))
        in_maps.append({
            "xT": xT_c, "wqk": wqk_c, "wv": wv_c, "wp": wp_c,
            "bqk": bqk_c, "tri": tri_np,
        })

    res = run_bass_kernel_spmd(nc, in_maps, list(range(NCORES)))
    _last_results = res
    outs = [res.results[c]["out"] for c in range(NCORES)]
    # v-bias: softmax rows sum to 1, so att @ (xWv + bv) = att @ (xWv) + bv;
    # its projection (bv @ W_proj) plus b_proj are added here, exactly.
    bv = b_attn[2 * N:3 * N]
    extra = bv @ W_proj + b_proj
    y = np.stack([outs[2 * b] + outs[2 * b + 1] for b in range(B)])
    return (y + extra[None, None, :]).astype(np.float32)
